# revision 9
# baseline (speedup 1.0000x reference)
"""GIN message-passing (2 GINConv + 2 linear) on 8 TRN2 NeuronCores.

Single pipelined phase (v2):
  - dst-node sharding (12500/core), dst tiles of 128; edges per core
    grouped L1 by (tile, src-quarter of x table), L2 by (tile, AG slice).
  - Gathers via gpsimd.dma_gather (256B rows), <=1024 idx/call, rr over
    4 SWDGE queues (queues generate descriptors in parallel, ~2ns/desc
    aggregate); scratch ring raised to 2048 descs/queue so one call can
    be generated while the previous drains.
  - Aggregation via one-hot matmuls: aggT[ch,dst] += Xe[slots,ch].T @ OH.
  - L1 per tile: z = relu((x+agg)@W1+b1)@W2; z rows (64ch + 64 zero pad,
    fp16) land in z_shard; sliced AllGather (NS=5 slices of 2500 rows)
    into Shared z_full [slice][core][row].
  - L2 work for slice s is emitted ~2 L1 batches after AG_s is issued, so
    its gathers+matmuls overlap the remainder of phase 1. Per (batch,
    slice, tile): psum agg -> DVE-add into l2acc (fp16 [64, SHARD]).
  - Per-tile epilogue after the last slice: h2 = relu(zT + l2acc + b2),
    h3 = relu(h2@W3+b3), out = h3@W4+b4 -> DMA per tile.
  - xT pre-tiled on host ([NT,128,128] fp16) so per-tile loads are one
    contiguous descriptor instead of 128 strided ones.
"""

import numpy as np

P = 128
CALL_MAX = 1024  # idx per dma_gather call (ring 2048/queue -> 1 gen-ahead)


class Cfg:
    def __init__(self, n_nodes, n_edges, in_ch, hid, n_cores, t_b, n_slices):
        self.N = n_nodes
        self.E = n_edges
        self.CH = in_ch
        self.H = hid
        self.NCORE = n_cores
        self.SHARD = n_nodes // n_cores
        self.NT = -(-self.SHARD // P)
        self.NQ = 4  # x16 table groups (quarters)
        self.QS = n_nodes // 4
        assert self.QS <= 32768
        self.NS = n_slices  # z table groups (AG slices)
        assert self.SHARD % n_slices == 0
        self.SROW = self.SHARD // n_slices
        assert self.SROW * n_cores <= 32768
        self.T_B = t_b
        self.NB = -(-self.NT // t_b)


FULL = Cfg(100000, 1600000, 128, 64, 8, 3, 5)


def _r128(a):
    return ((a + 127) // 128) * 128


class LayerPlan:
    """Gather schedule for one layer: per-(tile, group) budgets, batched
    layout, per-core idx/dstloc arrays."""

    def __init__(self, cfg, n_groups, per_core_edges, group_major=False):
        c = cfg
        self.cfg = c
        self.NG = n_groups
        counts = np.zeros((c.NCORE, c.NT, n_groups), dtype=np.int64)
        self.groups = []
        for ci in range(c.NCORE):
            t, grp, lidx, dl = per_core_edges[ci]
            key = t * n_groups + grp
            order = np.argsort(key, kind="stable")
            key = key[order]
            lidx = lidx[order].astype(np.int16)
            dl = dl[order].astype(np.int16)
            cnt = np.bincount(key, minlength=c.NT * n_groups).reshape(
                c.NT, n_groups
            )
            counts[ci] = cnt
            bounds = np.concatenate([[0], np.cumsum(cnt.reshape(-1))])
            g = {}
            for tt in range(c.NT):
                for qq in range(n_groups):
                    k = tt * n_groups + qq
                    lo, hi = bounds[k], bounds[k + 1]
                    if hi > lo:
                        g[(tt, qq)] = (lidx[lo:hi], dl[lo:hi])
            self.groups.append(g)

        # x128 so every tile's segment is chunk-aligned and matmuls have
        # full K=128
        self.B = _r128(counts.max(axis=0))  # [NT, NG]
        self.batches = []
        cgo = 0
        igo = 0
        for b in range(c.NB):
            tiles = list(range(b * c.T_B, min((b + 1) * c.T_B, c.NT)))
            quarters = []
            o = 0
            ig = 0
            for q in range(n_groups):
                segs = []
                s0 = 0
                for t in tiles:
                    btq = int(self.B[t, q])
                    if btq == 0:
                        continue
                    lo, hi = s0, s0 + btq
                    for ch in range(lo // P, (hi - 1) // P + 1):
                        segs.append((t, ch))
                    s0 = hi
                tot = s0
                nch = tot // P
                quarters.append(
                    dict(tot=tot, nch=nch, o=o, icols=tot // 16, ig=ig,
                         segs=segs)
                )
                o += nch
                ig += tot // 16
            self.batches.append(
                dict(tiles=tiles, quarters=quarters, cgo=cgo, igo=igo, nch=o)
            )
            cgo += o
            igo += ig
        self.CGOT = cgo
        self.IGOT = igo
        self.group_major = group_major
        if group_major:
            # region per group: [icols of all batches][dstloc of all batches]
            self.GM_IW = [sum(b["quarters"][q]["icols"] for b in self.batches)
                          for q in range(n_groups)]
            self.GM_CW = [sum(b["quarters"][q]["nch"] for b in self.batches)
                          for q in range(n_groups)]
            self.GM_I = np.concatenate([[0], np.cumsum(self.GM_IW)])
            self.GM_C = np.concatenate([[0], np.cumsum(self.GM_CW)])
            ci_ = [0] * n_groups
            cc_ = [0] * n_groups
            for b in self.batches:
                for q, qq in enumerate(b["quarters"]):
                    qq["gio"] = ci_[q]  # offset within group-q icol region
                    qq["gco"] = cc_[q]
                    ci_[q] += qq["icols"]
                    cc_[q] += qq["nch"]

    def core_arrays(self, ci):
        c = self.cfg
        idx16 = np.zeros((P, self.IGOT), dtype=np.int16)
        dstloc = np.full((P, self.CGOT), -1.0, dtype=np.float16)
        g = self.groups[ci]
        for b in self.batches:
            for q, qq in enumerate(b["quarters"]):
                tot = qq["tot"]
                if tot == 0:
                    continue
                sl_full = np.zeros(tot, dtype=np.int16)
                dl_full = np.full(tot, -1, dtype=np.int16)
                s0 = 0
                for t in b["tiles"]:
                    btq = int(self.B[t, q])
                    if btq == 0:
                        continue
                    if (t, q) in g:
                        sl, dl = g[(t, q)]
                        n = len(sl)
                        sl_full[s0 : s0 + n] = sl
                        dl_full[s0 : s0 + n] = dl
                    s0 += btq
                blk = sl_full.reshape(tot // 16, 16).T
                if self.group_major:
                    icol = int(self.GM_I[q]) + qq["gio"]
                    col0 = int(self.GM_C[q]) + qq["gco"]
                else:
                    icol = b["igo"] + qq["ig"]
                    col0 = b["cgo"] + qq["o"]
                idx16[:, icol : icol + tot // 16] = np.tile(blk, (8, 1))
                nch = qq["nch"]
                dstloc[:, col0 : col0 + nch] = (
                    dl_full.reshape(nch, P).T.astype(np.float16)
                )
        return idx16, dstloc


class Plan:
    def __init__(self, cfg, src, dst):
        c = cfg
        self.cfg = c
        core = dst // c.SHARD
        e1, e2 = [], []
        for ci in range(c.NCORE):
            m = core == ci
            s = src[m]
            d = dst[m] - ci * c.SHARD
            t = d // P
            dl = d % P
            e1.append((t, s // c.QS, s % c.QS, dl))
            # z_full layout: [slice][core][row]
            sc = s // c.SHARD
            sr = s % c.SHARD
            sl = sr // c.SROW
            lidx = sc * c.SROW + (sr - sl * c.SROW)
            e2.append((t, sl, lidx, dl))
        self.L1 = LayerPlan(cfg, c.NQ, e1)
        self.L2 = LayerPlan(cfg, c.NS, e2, group_major=True)

    def core_arrays(self, ci):
        i1, d1 = self.L1.core_arrays(ci)
        i2, d2 = self.L2.core_arrays(ci)
        return np.concatenate(
            [i1, d1.view(np.int16), i2, d2.view(np.int16)], axis=1
        )


def _build(plan):
    import concourse.tile as tile
    from concourse import bacc, mybir

    c = plan.cfg
    f16, f32, i16 = mybir.dt.float16, mybir.dt.float32, mybir.dt.int16
    CH, H, QS, SHARD, NS, SROW = c.CH, c.H, c.QS, c.SHARD, c.NS, c.SROW
    L1, L2 = plan.L1, plan.L2
    PKW = L1.IGOT + L1.CGOT + L2.IGOT + L2.CGOT

    nc = bacc.Bacc(
        "TRN2", target_bir_lowering=False, debug=False, num_devices=c.NCORE,
        num_swdge_queues=4, dynamic_dma_scratch_size=49152,
    )
    x16 = nc.dram_tensor("x16", [c.N, CH], f16, kind="ExternalInput")
    xTt = nc.dram_tensor("xTt", [c.NT * CH, P], f16, kind="ExternalInput")
    pk = nc.dram_tensor("pk", [P, PKW], i16, kind="ExternalInput")
    iota = nc.dram_tensor("iota", [P, P], f16, kind="ExternalInput")
    ident = nc.dram_tensor("ident", [H, H], f16, kind="ExternalInput")
    w1 = nc.dram_tensor("w1", [CH, H], f16, kind="ExternalInput")
    w2 = nc.dram_tensor("w2", [H, H], f16, kind="ExternalInput")
    w3 = nc.dram_tensor("w3", [H, 16], f16, kind="ExternalInput")
    w4 = nc.dram_tensor("w4", [16, 1], f16, kind="ExternalInput")
    b1 = nc.dram_tensor("b1", [H, 1], f32, kind="ExternalInput")
    b2 = nc.dram_tensor("b2", [H, 1], f32, kind="ExternalInput")
    b3 = nc.dram_tensor("b3", [16, 1], f32, kind="ExternalInput")
    b4v = nc.dram_tensor("b4v", [1, 1], f32, kind="ExternalInput")
    out = nc.dram_tensor("out", [1, SHARD], f32, kind="ExternalOutput")

    with tile.TileContext(nc) as tc:
        with (
            tc.tile_pool(name="const", bufs=1) as cp,
            tc.tile_pool(name="persist", bufs=1) as pp,
            tc.tile_pool(name="dram", bufs=1, space="DRAM") as dp,
            tc.tile_pool(name="l1g", bufs=2) as gp1,
            tc.tile_pool(name="l2g", bufs=3) as gp2,
            tc.tile_pool(name="oh1", bufs=2) as ohp1,
            tc.tile_pool(name="oh2", bufs=3) as ohp2,
            tc.tile_pool(name="sm", bufs=3) as sm,
            tc.tile_pool(name="pks", bufs=3) as pks,
            tc.tile_pool(name="l1ps", bufs=c.T_B, space="PSUM") as psa,
            tc.tile_pool(name="pst", bufs=2, space="PSUM") as pst,
            tc.tile_pool(name="l2ps", bufs=c.T_B, space="PSUM") as ps2,
        ):
            def load_const(name, t, shape, dt):
                sb = cp.tile(shape, dt, name=name + "_sb")
                nc.sync.dma_start(out=sb[:], in_=t[:, :])
                return sb

            iota_sb = load_const("iota", iota, [P, P], f16)
            id_sb = load_const("id", ident, [H, H], f16)
            w1_sb = load_const("w1", w1, [CH, H], f16)
            w2_sb = load_const("w2", w2, [H, H], f16)
            w3_sb = load_const("w3", w3, [H, 16], f16)
            w4_sb = load_const("w4", w4, [16, 1], f16)
            b1_sb = load_const("b1", b1, [H, 1], f32)
            b2_sb = load_const("b2", b2, [H, 1], f32)
            b3_sb = load_const("b3", b3, [16, 1], f32)
            b4_sb = load_const("b4", b4v, [1, 1], f32)

            zT_sh = pp.tile([H, SHARD], f16)
            l2acc = pp.tile([H, SHARD], f16)
            nc.vector.memset(l2acc[:], 0)
            z_shard = dp.tile([SHARD, P], f16)
            SR8_ = SROW * c.NCORE
            z_slices = [
                dp.tile([SR8_, P], f16, addr_space="Shared",
                        name=f"z_sl{s}")
                for s in range(NS)
            ]

            relu = mybir.ActivationFunctionType.Relu
            qrr = [0]

            def tile_cols(t):
                return min(P, SHARD - t * P)

            def stage_batch_pk(lp, b, pk_base):
                """One staging tile for a whole L1 batch (all quarters)."""
                icols = sum(q["icols"] for q in b["quarters"])
                nch = b["nch"]
                pk_sb = pks.tile([P, icols + nch], i16, tag="pk", name="pk")
                nc.sync.dma_start(
                    out=pk_sb[:, :icols],
                    in_=pk[:, pk_base + b["igo"]:
                           pk_base + b["igo"] + icols],
                )
                nc.sync.dma_start(
                    out=pk_sb[:, icols:],
                    in_=pk[:, pk_base + lp.IGOT + b["cgo"]:
                           pk_base + lp.IGOT + b["cgo"] + nch],
                )
                return pk_sb, icols

            def gather_calls(gbuf, go, table, t0, t1, elem, idx_ap, tot):
                for s0 in range(0, tot, CALL_MAX):
                    n = min(CALL_MAX, tot - s0)
                    c0 = go + s0 // P
                    nc.gpsimd.dma_gather(
                        gbuf[:, c0 : c0 + n // P, :],
                        table[t0:t1, :],
                        idx_ap[:, s0 // 16 : (s0 + n) // 16],
                        n, n, elem,
                        queue_num=qrr[0] % 4,
                    )
                    qrr[0] += 1

            def onehot(dl_sb, nch, pool):
                oh = pool.tile([P, nch, P], f16, tag="oh", name="oh")
                nc.vector.tensor_tensor(
                    out=oh[:],
                    in0=dl_sb.unsqueeze(2).to_broadcast([P, nch, P]),
                    in1=iota_sb[:].unsqueeze(1).to_broadcast([P, nch, P]),
                    op=mybir.AluOpType.is_equal,
                )
                return oh

            # ---- L1 batch emission ----
            def emit_l1_batch(b):
                if b["nch"] == 0:
                    return
                # global chain extents per tile across quarters
                first, last = {}, {}
                for q, qq in enumerate(b["quarters"]):
                    for (t, ch) in qq["segs"]:
                        first.setdefault(t, (q, ch))
                        last[t] = (q, ch)
                aggs = {
                    t: psa.tile([CH, P], f32, tag="agg1", name=f"agg1_{t}")
                    for t in b["tiles"] if t in first
                }
                g1 = gp1.tile([P, b["nch"], CH], f16, tag="g1", name="g1")
                pk_sb, icols_b = stage_batch_pk(L1, b, 0)
                for q, qq in enumerate(b["quarters"]):
                    if qq["tot"] == 0:
                        continue
                    gather_calls(
                        g1, qq["o"], x16, q * QS, (q + 1) * QS, CH,
                        pk_sb[:, qq["ig"] : qq["ig"] + qq["icols"]],
                        qq["tot"])
                    dl = pk_sb[:, icols_b + qq["o"]:
                               icols_b + qq["o"] + qq["nch"]].bitcast(f16)
                    oh = onehot(dl, qq["nch"], ohp1)
                    for (t, ch) in qq["segs"]:
                        nc.tensor.matmul(
                            out=aggs[t][:],
                            lhsT=g1[:, qq["o"] + ch : qq["o"] + ch + 1, :],
                            rhs=oh[:, ch : ch + 1, :],
                            start=(first[t] == (q, ch)),
                            stop=(last[t] == (q, ch)),
                        )
                for t in b["tiles"]:
                    tw = tile_cols(t)
                    xT_sb = sm.tile([CH, P], f16, tag="xt", name="xt")
                    nc.scalar.dma_start(
                        out=xT_sb[:], in_=xTt[t * CH : (t + 1) * CH, :])
                    sT = sm.tile([CH, P], f16, tag="st", name="st")
                    if t in aggs:
                        nc.vector.tensor_add(
                            out=sT[:, :tw], in0=aggs[t][:, :tw],
                            in1=xT_sb[:, :tw],
                        )
                    else:
                        nc.vector.tensor_copy(out=sT[:, :tw],
                                              in_=xT_sb[:, :tw])
                    h1p = pst.tile([H, P], f32, tag="mlp", name="h1p")
                    nc.tensor.matmul(
                        out=h1p[:, :tw], lhsT=w1_sb[:], rhs=sT[:, :tw],
                        start=True, stop=True,
                    )
                    h1f = sm.tile([H, P], f16, tag="h1f", name="h1f")
                    nc.scalar.activation(
                        out=h1f[:, :tw], in_=h1p[:, :tw], func=relu,
                        bias=b1_sb[:],
                    )
                    zp = pst.tile([H, P], f32, tag="mlp", name="zp")
                    nc.tensor.matmul(
                        out=zp[:, :tw], lhsT=w2_sb[:], rhs=h1f[:, :tw],
                        start=True, stop=True,
                    )
                    nc.vector.tensor_copy(
                        out=zT_sh[:, t * P : t * P + tw], in_=zp[:, :tw]
                    )
                    zf = sm.tile([H, P], f16, tag="zf", name="zf")
                    nc.vector.tensor_copy(out=zf[:, :tw], in_=zp[:, :tw])
                    ztr = pst.tile([P, H], f16, tag="mlp", name="ztr")
                    nc.tensor.transpose(
                        out=ztr[:tw, :], in_=zf[:, :tw], identity=id_sb[:]
                    )
                    zr = sm.tile([P, P], f16, tag="zr", name="zr")
                    nc.vector.memset(zr[:, H:], 0)
                    nc.vector.tensor_copy(out=zr[:tw, :H], in_=ztr[:tw, :])
                    nc.scalar.dma_start(
                        out=z_shard[t * P : t * P + tw, :], in_=zr[:tw, :]
                    )

            # ---- AG ----
            ag_row = [0]

            def issue_ag(r1):
                issued = []
                r1 = min(r1, SHARD)
                while ag_row[0] + SROW <= r1:
                    r0 = ag_row[0]
                    rs = r0 + SROW
                    issued.append(r0 // SROW)
                    nc.gpsimd.collective_compute(
                        "AllGather",
                        mybir.AluOpType.bypass,
                        replica_groups=[list(range(c.NCORE))],
                        ins=[z_shard[r0:rs, :].opt()],
                        outs=[z_slices[r0 // SROW][:, :].opt()],
                    )
                    ag_row[0] = rs
                return issued

            # ---- L2 unit emission: (batch, slice) ----
            pk2 = L1.IGOT + L1.CGOT
            SR8 = SROW * c.NCORE
            slice_pk = {}

            def stage_slice_pk(s):
                iw, cw = L2.GM_IW[s], L2.GM_CW[s]
                sb = pks.tile([P, iw + cw], i16, tag="pk2", name="pk2")
                nc.sync.dma_start(
                    out=sb[:, :iw],
                    in_=pk[:, pk2 + int(L2.GM_I[s]):
                           pk2 + int(L2.GM_I[s]) + iw],
                )
                nc.sync.dma_start(
                    out=sb[:, iw:],
                    in_=pk[:, pk2 + L2.IGOT + int(L2.GM_C[s]):
                           pk2 + L2.IGOT + int(L2.GM_C[s]) + cw],
                )
                slice_pk[s] = (sb, iw)

            def emit_l2_unit(lb, s):
                b = L2.batches[lb]
                qq = b["quarters"][s]
                if qq["tot"] == 0:
                    return
                sb, iw = slice_pk[s]
                g2 = gp2.tile([P, qq["nch"], P], f16, tag="g2", name="g2")
                gather_calls(
                    g2, 0, z_slices[s], 0, SR8, P,
                    sb[:, qq["gio"] : qq["gio"] + qq["icols"]], qq["tot"])
                dl = sb[:, iw + qq["gco"]:
                        iw + qq["gco"] + qq["nch"]].bitcast(f16)
                oh = onehot(dl, qq["nch"], ohp2)
                first, last, aggs = {}, {}, {}
                for (t, ch) in qq["segs"]:
                    first.setdefault(t, ch)
                    last[t] = ch
                for t in b["tiles"]:
                    if t in first:
                        aggs[t] = ps2.tile([P, P], f32, tag="agg2",
                                           name=f"agg2_{t}")
                for (t, ch) in qq["segs"]:
                    nc.tensor.matmul(
                        out=aggs[t][:],
                        lhsT=g2[:, ch : ch + 1, :],
                        rhs=oh[:, ch : ch + 1, :],
                        start=(first[t] == ch),
                        stop=(last[t] == ch),
                    )
                for t, a in aggs.items():
                    tw = tile_cols(t)
                    nc.vector.tensor_add(
                        out=l2acc[:, t * P : t * P + tw],
                        in0=l2acc[:, t * P : t * P + tw],
                        in1=a[:H, :tw],
                    )

            def emit_epilogue(t):
                tw = tile_cols(t)
                t2 = sm.tile([H, P], f32, tag="t2", name="t2")
                nc.vector.tensor_add(
                    out=t2[:, :tw], in0=l2acc[:, t * P : t * P + tw],
                    in1=zT_sh[:, t * P : t * P + tw],
                )
                h2f = sm.tile([H, P], f16, tag="h2f", name="h2f")
                nc.scalar.activation(
                    out=h2f[:, :tw], in_=t2[:, :tw], func=relu,
                    bias=b2_sb[:],
                )
                h3p = pst.tile([16, P], f32, tag="mlp", name="h3p")
                nc.tensor.matmul(
                    out=h3p[:, :tw], lhsT=w3_sb[:], rhs=h2f[:, :tw],
                    start=True, stop=True,
                )
                h3f = sm.tile([16, P], f16, tag="h3f", name="h3f")
                nc.scalar.activation(
                    out=h3f[:, :tw], in_=h3p[:, :tw], func=relu,
                    bias=b3_sb[:],
                )
                op_ = pst.tile([1, P], f32, tag="mlp", name="op_")
                nc.tensor.matmul(
                    out=op_[:, :tw], lhsT=w4_sb[:], rhs=h3f[:, :tw],
                    start=True, stop=True,
                )
                ot = sm.tile([1, P], f32, tag="ot", name="ot")
                nc.vector.scalar_tensor_tensor(
                    out=ot[:, :tw],
                    in0=op_[:, :tw],
                    scalar=1.0,
                    in1=b4_sb[:].to_broadcast([1, tw]),
                    op0=mybir.AluOpType.mult,
                    op1=mybir.AluOpType.add,
                )
                nc.scalar.dma_start(
                    out=out[:, t * P : t * P + tw], in_=ot[:, :tw]
                )

            # ---- main pipelined loop ----
            l2_ready = []   # (lb, s) units whose slice has been AG'd
            l2_delay = []   # freshly issued slices, delayed 2 L1 batches
            n_l1 = len(L1.batches)
            n_lb = len(L2.batches)
            total_units = n_lb * NS

            def promote():
                for item in l2_delay:
                    item[0] -= 1
                while l2_delay and l2_delay[0][0] <= 0:
                    _, s = l2_delay.pop(0)
                    for lb in range(n_lb):
                        l2_ready.append((lb, s))

            emitted = 0
            for bi, b in enumerate(L1.batches):
                emit_l1_batch(b)
                done_rows = (b["tiles"][-1] + 1) * P
                for s in issue_ag((done_rows // SROW) * SROW):
                    stage_slice_pk(s)
                    l2_delay.append([99, s])
                promote()
                if bi < n_l1 - 1:
                    k = 0
                    while l2_ready and k < 6:
                        lb, s = l2_ready.pop(0)
                        emit_l2_unit(lb, s)
                        emitted += 1
                        k += 1
            for s in issue_ag(SHARD):
                stage_slice_pk(s)
                l2_delay.append([0, s])
            for item in l2_delay:
                for lb in range(n_lb):
                    l2_ready.append((lb, item[1]))
            for lb, s in l2_ready:
                emit_l2_unit(lb, s)
                emitted += 1
                if s == NS - 1:
                    for t in L2.batches[lb]["tiles"]:
                        emit_epilogue(t)
            assert emitted == total_units, (emitted, total_units)
    nc.compile()
    return nc


def _in_maps(cfg, plan, x, W1, b1, W2, b2, W3, b3, W4, b4):
    c = cfg
    xTt_all = np.zeros((c.NCORE, c.NT * c.CH, P), dtype=np.float16)
    for ci in range(c.NCORE):
        xs = x[ci * c.SHARD : (ci + 1) * c.SHARD].astype(np.float16)
        for t in range(c.NT):
            blk = xs[t * P : (t + 1) * P]
            xTt_all[ci, t * c.CH : (t + 1) * c.CH, : blk.shape[0]] = blk.T
    common = dict(
        x16=x.astype(np.float16),
        iota=np.broadcast_to(np.arange(P, dtype=np.float16), (P, P)).copy(),
        ident=np.eye(c.H, dtype=np.float16),
        w1=W1.astype(np.float16),
        w2=W2.astype(np.float16),
        w3=W3.astype(np.float16),
        w4=W4.astype(np.float16),
        b1=b1.reshape(-1, 1).astype(np.float32),
        b2=b2.reshape(-1, 1).astype(np.float32),
        b3=b3.reshape(-1, 1).astype(np.float32),
        b4v=b4.reshape(1, 1).astype(np.float32),
    )
    in_maps = []
    for ci in range(c.NCORE):
        pk_a = plan.core_arrays(ci)
        in_maps.append(dict(common, pk=pk_a, xTt=xTt_all[ci]))
    return in_maps


def _run(cfg, plan, nc, x, W1, b1, W2, b2, W3, b3, W4, b4, **kw):
    from concourse.bass_utils import run_bass_kernel_spmd

    c = cfg
    in_maps = _in_maps(cfg, plan, x, W1, b1, W2, b2, W3, b3, W4, b4)
    res = run_bass_kernel_spmd(nc, in_maps, core_ids=list(range(c.NCORE)), **kw)
    outs = [res.results[ci]["out"].reshape(-1) for ci in range(c.NCORE)]
    return np.concatenate(outs).reshape(-1, 1).astype(np.float32), res


def kernel(x, edge_index, W1, b1, W2, b2, W3, b3, W4, b4):
    cfg = FULL
    x = np.asarray(x, dtype=np.float32)
    src = np.asarray(edge_index[0], dtype=np.int64)
    dst = np.asarray(edge_index[1], dtype=np.int64)
    plan = Plan(cfg, src, dst)
    nc = _build(plan)
    out, _ = _run(
        cfg, plan, nc, x,
        np.asarray(W1), np.asarray(b1), np.asarray(W2), np.asarray(b2),
        np.asarray(W3), np.asarray(b3), np.asarray(W4), np.asarray(b4),
    )
    return out


# revision 12
# speedup vs baseline: 1.2997x; 1.2997x over previous
"""GIN message-passing (2 GINConv layers + 2 linear) on 8 TRN2 NeuronCores.

Strategy (self-contained; shapes hardcoded for the 100k-node / 1.6M-edge
problem):
  - Shard dst nodes across 8 cores (12500 each). Each core owns the edges
    whose dst is in its shard; dst tiles of 128 nodes.
  - Per tile-batch, gather source-node rows with gpsimd.dma_gather (int16
    indices -> node table split into 4 groups <=32768 rows; <=1024 idx per
    call; calls round-robin over 4 SWDGE queues so all Q7 pairs work), then
    aggregate with one-hot matmuls: aggT[ch, dst] += Xe[slots, ch].T @ onehot.
  - Layer 1 gathers x in fp16 (256B rows, table replicated via inputs),
    computes z = relu((x + A@x)@W1 + b1) @ W2 per shard, writes z rows fp16
    (padded to 128 ch) to HBM; a SLICED AllGather (overlapped with phase 1)
    shares z: z_full layout is [slice][core][row] so each slice's AG output
    is contiguous.
  - Layer 2 gathers z rows (fp16 256B), h2 = relu(z_dst + A@z + b2),
    h3 = relu(h2@W3+b3), out = h3@W4+b4.
  - Per-(tile,group) slot budgets are static (max over cores, x128) with
    idx-0 padding masked by dstloc=-1 one-hots.
"""

import numpy as np

P = 128


class Cfg:
    def __init__(self, n_nodes, n_edges, in_ch, hid, n_cores, t_b, n_slices=4):
        self.N = n_nodes
        self.E = n_edges
        self.CH = in_ch
        self.H = hid
        self.NCORE = n_cores
        self.SHARD = n_nodes // n_cores
        self.NT = -(-self.SHARD // P)
        self.NQ = 4  # x16 table groups (quarters)
        self.QS = n_nodes // 4
        assert self.QS <= 32768
        self.NS = n_slices  # z table groups (AG slices)
        assert self.SHARD % n_slices == 0
        self.SROW = self.SHARD // n_slices  # shard rows per slice
        assert self.SROW * n_cores <= 32768
        self.T_B = t_b
        self.NB = -(-self.NT // t_b)


FULL = Cfg(100000, 1600000, 128, 64, 8, 4)


def _r128(a):
    return ((a + 127) // 128) * 128


class LayerPlan:
    """Gather schedule for one layer: per-(tile, group) budgets, batched
    chunk layout, per-core idx/dstloc arrays."""

    def __init__(self, cfg, n_groups, per_core_edges):
        # per_core_edges: [core] -> (t, grp, lidx, dl) int arrays
        c = cfg
        self.cfg = c
        self.NG = n_groups
        counts = np.zeros((c.NCORE, c.NT, n_groups), dtype=np.int64)
        self.groups = []
        for ci in range(c.NCORE):
            t, grp, lidx, dl = per_core_edges[ci]
            key = t * n_groups + grp
            order = np.argsort(key, kind="stable")
            key = key[order]
            lidx = lidx[order].astype(np.int16)
            dl = dl[order].astype(np.int16)
            cnt = np.bincount(key, minlength=c.NT * n_groups).reshape(
                c.NT, n_groups
            )
            counts[ci] = cnt
            bounds = np.concatenate([[0], np.cumsum(cnt.reshape(-1))])
            g = {}
            for tt in range(c.NT):
                for qq in range(n_groups):
                    k = tt * n_groups + qq
                    lo, hi = bounds[k], bounds[k + 1]
                    if hi > lo:
                        g[(tt, qq)] = (lidx[lo:hi], dl[lo:hi])
            self.groups.append(g)
        self.NGR = n_groups

        # x128 so every tile's segment is chunk-aligned (PE matmuls need
        # base partition 0) and every matmul has full K=128
        self.B = _r128(counts.max(axis=0))  # [NT, NG]
        self.batches = []
        cgo = 0
        igo = 0
        for b in range(c.NB):
            tiles = list(range(b * c.T_B, min((b + 1) * c.T_B, c.NT)))
            quarters = []
            o = 0
            for q in range(n_groups):
                segs = []
                s0 = 0
                for t in tiles:
                    btq = int(self.B[t, q])
                    if btq == 0:
                        continue
                    lo, hi = s0, s0 + btq
                    for ch in range(lo // P, (hi - 1) // P + 1):
                        segs.append((t, ch))
                    s0 = hi
                tot = s0
                nch = tot // P
                quarters.append(
                    dict(tot=tot, nch=nch, o=o, icols=tot // 16, segs=segs)
                )
                o += nch
            self.batches.append(
                dict(tiles=tiles, quarters=quarters, cgo=cgo, igo=igo, nch=o)
            )
            cgo += o
            igo += sum(qq["icols"] for qq in quarters)
        self.CGOT = cgo
        self.IGOT = igo

    def core_arrays(self, ci):
        c = self.cfg
        idx16 = np.zeros((P, self.IGOT), dtype=np.int16)
        dstloc = np.full((P, self.CGOT), -1.0, dtype=np.float16)
        cnt = np.zeros((c.NT, self.NGR), dtype=np.int32)
        g = self.groups[ci]
        for b in self.batches:
            icol = b["igo"]
            for q, qq in enumerate(b["quarters"]):
                tot = qq["tot"]
                if tot == 0:
                    continue
                sl_full = np.full(tot, -1, dtype=np.int16)
                dl_full = np.full(tot, -1, dtype=np.int16)
                s0 = 0
                for t in b["tiles"]:
                    btq = int(self.B[t, q])
                    if btq == 0:
                        continue
                    n = 0
                    if (t, q) in g:
                        sl, dl = g[(t, q)]
                        n = len(sl)
                        sl_full[s0 : s0 + n] = sl
                        dl_full[s0 : s0 + n] = dl
                    nv = max(n, min(16, btq))
                    sl_full[s0 + n : s0 + nv] = 0
                    cnt[t, q] = nv
                    s0 += btq
                blk = sl_full.reshape(tot // 16, 16).T
                idx16[:, icol : icol + tot // 16] = np.tile(blk, (8, 1))
                icol += tot // 16
                col0 = b["cgo"] + qq["o"]
                nch = qq["nch"]
                dstloc[:, col0 : col0 + nch] = (
                    dl_full.reshape(nch, P).T.astype(np.float16)
                )
        return idx16, dstloc, cnt


class Plan:
    def __init__(self, cfg, src, dst):
        c = cfg
        self.cfg = c
        core = dst // c.SHARD
        e1, e2 = [], []
        for ci in range(c.NCORE):
            m = core == ci
            s = src[m]
            d = dst[m] - ci * c.SHARD
            t = d // P
            dl = d % P
            # layer 1: x16 table grouped by quarters of node id
            e1.append((t, s // c.QS, s % c.QS, dl))
            # layer 2: z_full is [slice][core][row]; slice = shard-row/SROW
            sc = s // c.SHARD
            sr = s % c.SHARD
            sl = sr // c.SROW
            lidx = sc * c.SROW + (sr - sl * c.SROW)
            e2.append((t, sl, lidx, dl))
        self.L1 = LayerPlan(cfg, c.NQ, e1)
        self.L2 = LayerPlan(cfg, c.NS, e2)

    def core_arrays(self, ci):
        i1, d1, c1 = self.L1.core_arrays(ci)
        i2, d2, c2 = self.L2.core_arrays(ci)
        cnts = np.concatenate([c1.reshape(-1), c2.reshape(-1)])
        cnt16 = np.broadcast_to(
            cnts.astype("<i4").view("<i2"), (P, 2 * len(cnts))
        )
        return np.concatenate(
            [i1, d1.view(np.int16), i2, d2.view(np.int16), cnt16], axis=1
        )


def _build(plan):
    import concourse.tile as tile
    from concourse import bacc, mybir

    c = plan.cfg
    f16, f32, i16 = mybir.dt.float16, mybir.dt.float32, mybir.dt.int16
    CH, H, QS, SHARD = c.CH, c.H, c.QS, c.SHARD
    L1, L2 = plan.L1, plan.L2
    CNTW = 2 * (c.NT * (c.NQ + c.NS))
    PKW = L1.IGOT + L1.CGOT + L2.IGOT + L2.CGOT + CNTW
    i32 = mybir.dt.int32

    nc = bacc.Bacc(
        "TRN2", target_bir_lowering=False, debug=False, num_devices=c.NCORE,
        num_swdge_queues=4, dynamic_dma_scratch_size=32768,
    )
    x16 = nc.dram_tensor("x16", [c.N, CH], f16, kind="ExternalInput")
    xT = nc.dram_tensor("xT", [CH, SHARD], f32, kind="ExternalInput")
    pk = nc.dram_tensor("pk", [P, PKW], i16, kind="ExternalInput")
    iota = nc.dram_tensor("iota", [P, P], f16, kind="ExternalInput")
    ident = nc.dram_tensor("ident", [H, H], f16, kind="ExternalInput")
    w1 = nc.dram_tensor("w1", [CH, H], f16, kind="ExternalInput")
    w2 = nc.dram_tensor("w2", [H, H], f16, kind="ExternalInput")
    w3 = nc.dram_tensor("w3", [H, 16], f16, kind="ExternalInput")
    w4 = nc.dram_tensor("w4", [16, 1], f16, kind="ExternalInput")
    b1 = nc.dram_tensor("b1", [H, 1], f32, kind="ExternalInput")
    b2 = nc.dram_tensor("b2", [H, 1], f32, kind="ExternalInput")
    b3 = nc.dram_tensor("b3", [16, 1], f32, kind="ExternalInput")
    b4v = nc.dram_tensor("b4v", [1, 1], f32, kind="ExternalInput")
    out = nc.dram_tensor("out", [1, SHARD], f32, kind="ExternalOutput")

    with tile.TileContext(nc) as tc:
        with (
            tc.tile_pool(name="const", bufs=1) as cp,
            tc.tile_pool(name="persist", bufs=1) as pp,
            tc.tile_pool(name="dram", bufs=1, space="DRAM") as dp,
        ):
            def load_const(name, t, shape, dt):
                sb = cp.tile(shape, dt, name=name + "_sb")
                nc.sync.dma_start(out=sb[:], in_=t[:, :])
                return sb

            iota_sb = load_const("iota", iota, [P, P], f16)
            id_sb = load_const("id", ident, [H, H], f16)
            w1_sb = load_const("w1", w1, [CH, H], f16)
            w2_sb = load_const("w2", w2, [H, H], f16)
            w3_sb = load_const("w3", w3, [H, 16], f16)
            w4_sb = load_const("w4", w4, [16, 1], f16)
            b1_sb = load_const("b1", b1, [H, 1], f32)
            b2_sb = load_const("b2", b2, [H, 1], f32)
            b3_sb = load_const("b3", b3, [16, 1], f32)
            b4_sb = load_const("b4", b4v, [1, 1], f32)

            cnt_sb = pp.tile([P, CNTW], i16)
            nc.sync.dma_start(
                out=cnt_sb[:], in_=pk[:, PKW - CNTW : PKW])
            zT_sh = pp.tile([H, SHARD], f32)
            outT = pp.tile([1, SHARD], f32)
            z_shard = dp.tile([SHARD, P], f16)
            z_full = dp.tile([c.N, P], f16)  # [slice][core][row][ch]

            relu = mybir.ActivationFunctionType.Relu
            qrr = [0]  # round-robin SWDGE queue
            state = {}

            def tile_cols(t):
                return min(P, SHARD - t * P)

            def gather_batch(lp, b, pk_base, gbuf, table, tbase_of, elem,
                             cnt_base):
                sm = state["sm"]
                icols = sum(q["icols"] for q in b["quarters"])
                nch = b["nch"]
                pk_sb = sm.tile([P, icols + nch], i16, tag="pk")
                nc.sync.dma_start(
                    out=pk_sb[:, :icols],
                    in_=pk[:, pk_base + b["igo"] : pk_base + b["igo"] + icols],
                )
                nc.sync.dma_start(
                    out=pk_sb[:, icols:],
                    in_=pk[
                        :,
                        pk_base + lp.IGOT + b["cgo"] :
                        pk_base + lp.IGOT + b["cgo"] + nch,
                    ],
                )
                ic = 0
                for q, qq in enumerate(b["quarters"]):
                    if qq["tot"] == 0:
                        continue
                    t0, t1 = tbase_of(q)
                    s0 = 0
                    for t in b["tiles"]:
                        btq = int(lp.B[t, q])
                        if btq == 0:
                            continue
                        co = (cnt_base + t * lp.NGR + q) * 2
                        reg = nc.gpsimd.value_load(
                            cnt_sb[0:1, co : co + 2].bitcast(i32),
                            min_val=1, max_val=btq,
                        )
                        c0 = qq["o"] + s0 // P
                        nc.gpsimd.dma_gather(
                            gbuf[:, c0 : c0 + btq // P, :],
                            table[t0:t1, :],
                            pk_sb[:, ic + s0 // 16 : ic + (s0 + btq) // 16],
                            btq, reg, elem,
                            queue_num=qrr[0] % 4,
                        )
                        qrr[0] += 1
                        s0 += btq
                    ic += qq["icols"]
                return pk_sb[:, icols:].bitcast(f16)

            def onehot_and_agg(b, dl_sb, gbuf, ohp, psa, m_dim, tag):
                nch = b["nch"]
                oh = ohp.tile([P, nch, P], f16, tag="oh", name="oh")
                nc.vector.tensor_tensor(
                    out=oh[:],
                    in0=dl_sb.unsqueeze(2).to_broadcast([P, nch, P]),
                    in1=iota_sb[:].unsqueeze(1).to_broadcast([P, nch, P]),
                    op=mybir.AluOpType.is_equal,
                )
                mm = []
                for q, qq in enumerate(b["quarters"]):
                    for (t, ch) in qq["segs"]:
                        mm.append((t, qq["o"] + ch))
                first, last, aggs = {}, {}, {}
                for i, (t, ch) in enumerate(mm):
                    first.setdefault(t, i)
                    last[t] = i
                for t in b["tiles"]:
                    aggs[t] = psa.tile(
                        [m_dim, P], f32, tag=tag, name=f"{tag}_{t}"
                    )
                for i, (t, ch) in enumerate(mm):
                    nc.tensor.matmul(
                        out=aggs[t][:],
                        lhsT=gbuf[:, ch : ch + 1, :],
                        rhs=oh[:, ch : ch + 1, :],
                        start=(i == first[t]),
                        stop=(i == last[t]),
                    )
                return aggs

            # ---------------- phase 1 ----------------
            with (
                tc.tile_pool(name="l1g", bufs=2) as gp,
                tc.tile_pool(name="l1oh", bufs=2) as ohp,
                tc.tile_pool(name="l1sm", bufs=3) as sm,
                tc.tile_pool(name="l1ps", bufs=c.T_B + 1, space="PSUM") as psa,
                tc.tile_pool(name="l1pst", bufs=1, space="PSUM") as pst,
            ):
                state["sm"] = sm
                ag_row = [0]

                def issue_ag(r1):
                    # one collective per completed slice: the AG output
                    # (concat over cores) is contiguous only within a slice
                    r1 = min(r1, SHARD)
                    while ag_row[0] + c.SROW <= r1:
                        r0 = ag_row[0]
                        rs = r0 + c.SROW
                        nc.gpsimd.collective_compute(
                            "AllGather",
                            mybir.AluOpType.bypass,
                            replica_groups=[list(range(c.NCORE))],
                            ins=[z_shard[r0:rs, :].opt()],
                            outs=[
                                z_full[r0 * c.NCORE : rs * c.NCORE, :].opt()
                            ],
                        )
                        ag_row[0] = rs

                for b in L1.batches:
                    if b["nch"] == 0:
                        continue
                    g1 = gp.tile([P, b["nch"], CH], f16, tag="g1", name="g1")
                    dl_sb = gather_batch(
                        L1, b, 0, g1, x16,
                        lambda q: (q * QS, (q + 1) * QS), CH, 0,
                    )
                    aggs = onehot_and_agg(b, dl_sb, g1, ohp, psa, CH, "agg1")
                    for t in b["tiles"]:
                        tw = tile_cols(t)
                        xT_sb = sm.tile([CH, P], f32, tag="xt", name="xt")
                        nc.scalar.dma_start(
                            out=xT_sb[:, :tw], in_=xT[:, t * P : t * P + tw]
                        )
                        sT = sm.tile([CH, P], f16, tag="st", name="st")
                        nc.vector.tensor_add(
                            out=sT[:, :tw], in0=aggs[t][:, :tw],
                            in1=xT_sb[:, :tw],
                        )
                        h1p = pst.tile([H, P], f32, tag="h1", name="h1p")
                        nc.tensor.matmul(
                            out=h1p[:, :tw], lhsT=w1_sb[:], rhs=sT[:, :tw],
                            start=True, stop=True,
                        )
                        h1f = sm.tile([H, P], f16, tag="h1f", name="h1f")
                        nc.scalar.activation(
                            out=h1f[:, :tw], in_=h1p[:, :tw], func=relu,
                            bias=b1_sb[:],
                        )
                        zp = pst.tile([H, P], f32, tag="zp", name="zp")
                        nc.tensor.matmul(
                            out=zp[:, :tw], lhsT=w2_sb[:], rhs=h1f[:, :tw],
                            start=True, stop=True,
                        )
                        nc.vector.tensor_copy(
                            out=zT_sh[:, t * P : t * P + tw], in_=zp[:, :tw]
                        )
                        zf = sm.tile([H, P], f16, tag="zf", name="zf")
                        nc.vector.tensor_copy(out=zf[:, :tw], in_=zp[:, :tw])
                        ztr = pst.tile([P, H], f16, tag="ztr", name="ztr")
                        nc.tensor.transpose(
                            out=ztr[:tw, :], in_=zf[:, :tw], identity=id_sb[:]
                        )
                        zr = sm.tile([P, P], f16, tag="zr", name="zr")
                        nc.vector.memset(zr[:, H:], 0)
                        nc.vector.tensor_copy(out=zr[:tw, :H], in_=ztr[:tw, :])
                        nc.scalar.dma_start(
                            out=z_shard[t * P : t * P + tw, :], in_=zr[:tw, :]
                        )
                    done_rows = (b["tiles"][-1] + 1) * P
                    issue_ag((done_rows // c.SROW) * c.SROW)
                issue_ag(SHARD)

            # ---------------- phase 2 ----------------
            pk2 = L1.IGOT + L1.CGOT
            with (
                tc.tile_pool(name="l2g", bufs=2) as gp,
                tc.tile_pool(name="l2oh", bufs=2) as ohp,
                tc.tile_pool(name="l2sm", bufs=3) as sm,
                tc.tile_pool(name="l2ps", bufs=c.T_B + 1, space="PSUM") as psa,
                tc.tile_pool(name="l2pst", bufs=1, space="PSUM") as pst,
            ):
                state["sm"] = sm
                SR8 = c.SROW * c.NCORE
                for b in L2.batches:
                    if b["nch"] == 0:
                        continue
                    g2 = gp.tile([P, b["nch"], P], f16, tag="g2", name="g2")
                    dl_sb = gather_batch(
                        L2, b, pk2, g2, z_full,
                        lambda s: (s * SR8, (s + 1) * SR8), P,
                        c.NT * c.NQ,
                    )
                    aggs = onehot_and_agg(b, dl_sb, g2, ohp, psa, P, "agg2")
                    for t in b["tiles"]:
                        tw = tile_cols(t)
                        t2 = sm.tile([H, P], f32, tag="t2", name="t2")
                        nc.vector.tensor_add(
                            out=t2[:, :tw], in0=aggs[t][:H, :tw],
                            in1=zT_sh[:, t * P : t * P + tw],
                        )
                        h2f = sm.tile([H, P], f16, tag="h2f", name="h2f")
                        nc.scalar.activation(
                            out=h2f[:, :tw], in_=t2[:, :tw], func=relu,
                            bias=b2_sb[:],
                        )
                        h3p = pst.tile([16, P], f32, tag="h3", name="h3p")
                        nc.tensor.matmul(
                            out=h3p[:, :tw], lhsT=w3_sb[:], rhs=h2f[:, :tw],
                            start=True, stop=True,
                        )
                        h3f = sm.tile([16, P], f16, tag="h3f", name="h3f")
                        nc.scalar.activation(
                            out=h3f[:, :tw], in_=h3p[:, :tw], func=relu,
                            bias=b3_sb[:],
                        )
                        op_ = pst.tile([1, P], f32, tag="op", name="op_")
                        nc.tensor.matmul(
                            out=op_[:, :tw], lhsT=w4_sb[:], rhs=h3f[:, :tw],
                            start=True, stop=True,
                        )
                        nc.vector.scalar_tensor_tensor(
                            out=outT[:, t * P : t * P + tw],
                            in0=op_[:, :tw],
                            scalar=1.0,
                            in1=b4_sb[:].to_broadcast([1, tw]),
                            op0=mybir.AluOpType.mult,
                            op1=mybir.AluOpType.add,
                        )
            nc.sync.dma_start(out=out[:, :], in_=outT[:])
    nc.compile()
    return nc


def _in_maps(cfg, plan, x, W1, b1, W2, b2, W3, b3, W4, b4):
    c = cfg
    common = dict(
        x16=x.astype(np.float16),
        iota=np.broadcast_to(np.arange(P, dtype=np.float16), (P, P)).copy(),
        ident=np.eye(c.H, dtype=np.float16),
        w1=W1.astype(np.float16),
        w2=W2.astype(np.float16),
        w3=W3.astype(np.float16),
        w4=W4.astype(np.float16),
        b1=b1.reshape(-1, 1).astype(np.float32),
        b2=b2.reshape(-1, 1).astype(np.float32),
        b3=b3.reshape(-1, 1).astype(np.float32),
        b4v=b4.reshape(1, 1).astype(np.float32),
    )
    in_maps = []
    for ci in range(c.NCORE):
        pk_a = plan.core_arrays(ci)
        xT_a = np.ascontiguousarray(
            x[ci * c.SHARD : (ci + 1) * c.SHARD].T.astype(np.float32)
        )
        in_maps.append(dict(common, pk=pk_a, xT=xT_a))
    return in_maps


def _run(cfg, plan, nc, x, W1, b1, W2, b2, W3, b3, W4, b4, **kw):
    from concourse.bass_utils import run_bass_kernel_spmd

    c = cfg
    in_maps = _in_maps(cfg, plan, x, W1, b1, W2, b2, W3, b3, W4, b4)
    res = run_bass_kernel_spmd(nc, in_maps, core_ids=list(range(c.NCORE)), **kw)
    outs = [res.results[ci]["out"].reshape(-1) for ci in range(c.NCORE)]
    return np.concatenate(outs).reshape(-1, 1).astype(np.float32), res


def kernel(x, edge_index, W1, b1, W2, b2, W3, b3, W4, b4):
    cfg = FULL
    x = np.asarray(x, dtype=np.float32)
    src = np.asarray(edge_index[0], dtype=np.int64)
    dst = np.asarray(edge_index[1], dtype=np.int64)
    plan = Plan(cfg, src, dst)
    nc = _build(plan)
    out, _ = _run(
        cfg, plan, nc, x,
        np.asarray(W1), np.asarray(b1), np.asarray(W2), np.asarray(b2),
        np.asarray(W3), np.asarray(b3), np.asarray(W4), np.asarray(b4),
    )
    return out



# revision 13
# speedup vs baseline: 1.3396x; 1.0307x over previous
"""GIN message-passing (2 GINConv layers + 2 linear) on 8 TRN2 NeuronCores.

Strategy (self-contained; shapes hardcoded for the 100k-node / 1.6M-edge
problem):
  - Shard dst nodes across 8 cores (12500 each). Each core owns the edges
    whose dst is in its shard; dst tiles of 128 nodes.
  - Per tile-batch, gather source-node rows with gpsimd.dma_gather (int16
    indices -> node table split into 4 groups <=32768 rows; <=1024 idx per
    call; calls round-robin over 4 SWDGE queues so all Q7 pairs work), then
    aggregate with one-hot matmuls: aggT[ch, dst] += Xe[slots, ch].T @ onehot.
  - Layer 1 gathers x in fp16 (256B rows, table replicated via inputs),
    computes z = relu((x + A@x)@W1 + b1) @ W2 per shard, writes z rows fp16
    (padded to 128 ch) to HBM; a SLICED AllGather (overlapped with phase 1)
    shares z: z_full layout is [slice][core][row] so each slice's AG output
    is contiguous.
  - Layer 2 gathers z rows (fp16 256B), h2 = relu(z_dst + A@z + b2),
    h3 = relu(h2@W3+b3), out = h3@W4+b4.
  - Per-(tile,group) slot budgets are static (max over cores, x128) with
    idx-0 padding masked by dstloc=-1 one-hots.
"""

import numpy as np

P = 128


class Cfg:
    def __init__(self, n_nodes, n_edges, in_ch, hid, n_cores, t_b, n_slices=4):
        self.N = n_nodes
        self.E = n_edges
        self.CH = in_ch
        self.H = hid
        self.NCORE = n_cores
        self.SHARD = n_nodes // n_cores
        self.NT = -(-self.SHARD // P)
        self.NQ = 4  # x16 table groups (quarters)
        self.QS = n_nodes // 4
        assert self.QS <= 32768
        self.NS = n_slices  # z table groups (AG slices)
        assert self.SHARD % n_slices == 0
        self.SROW = self.SHARD // n_slices  # shard rows per slice
        assert self.SROW * n_cores <= 32768
        self.T_B = t_b
        self.NB = -(-self.NT // t_b)


FULL = Cfg(100000, 1600000, 128, 64, 8, 4)


def _r128(a):
    return ((a + 127) // 128) * 128


class LayerPlan:
    """Gather schedule for one layer: per-(tile, group) budgets, batched
    chunk layout, per-core idx/dstloc arrays."""

    def __init__(self, cfg, n_groups, per_core_edges):
        # per_core_edges: [core] -> (t, grp, lidx, dl) int arrays
        c = cfg
        self.cfg = c
        self.NG = n_groups
        counts = np.zeros((c.NCORE, c.NT, n_groups), dtype=np.int64)
        self.groups = []
        for ci in range(c.NCORE):
            t, grp, lidx, dl = per_core_edges[ci]
            key = t * n_groups + grp
            order = np.argsort(key, kind="stable")
            key = key[order]
            lidx = lidx[order].astype(np.int16)
            dl = dl[order].astype(np.int16)
            cnt = np.bincount(key, minlength=c.NT * n_groups).reshape(
                c.NT, n_groups
            )
            counts[ci] = cnt
            bounds = np.concatenate([[0], np.cumsum(cnt.reshape(-1))])
            g = {}
            for tt in range(c.NT):
                for qq in range(n_groups):
                    k = tt * n_groups + qq
                    lo, hi = bounds[k], bounds[k + 1]
                    if hi > lo:
                        g[(tt, qq)] = (lidx[lo:hi], dl[lo:hi])
            self.groups.append(g)
        self.NGR = n_groups

        # x128 so every tile's segment is chunk-aligned (PE matmuls need
        # base partition 0) and every matmul has full K=128
        self.B = _r128(counts.max(axis=0))  # [NT, NG]
        self.batches = []
        cgo = 0
        igo = 0
        for b in range(c.NB):
            tiles = list(range(b * c.T_B, min((b + 1) * c.T_B, c.NT)))
            quarters = []
            o = 0
            for q in range(n_groups):
                segs = []
                s0 = 0
                for t in tiles:
                    btq = int(self.B[t, q])
                    if btq == 0:
                        continue
                    lo, hi = s0, s0 + btq
                    for ch in range(lo // P, (hi - 1) // P + 1):
                        segs.append((t, ch))
                    s0 = hi
                tot = s0
                nch = tot // P
                quarters.append(
                    dict(tot=tot, nch=nch, o=o, icols=tot // 16, segs=segs)
                )
                o += nch
            self.batches.append(
                dict(tiles=tiles, quarters=quarters, cgo=cgo, igo=igo, nch=o)
            )
            cgo += o
            igo += sum(qq["icols"] for qq in quarters)
        self.CGOT = cgo
        self.IGOT = igo

    def core_arrays(self, ci):
        c = self.cfg
        idx16 = np.zeros((P, self.IGOT), dtype=np.int16)
        dstloc = np.full((P, self.CGOT), -1.0, dtype=np.float16)
        cnt = np.zeros((c.NT, self.NGR), dtype=np.int32)
        g = self.groups[ci]
        for b in self.batches:
            icol = b["igo"]
            for q, qq in enumerate(b["quarters"]):
                tot = qq["tot"]
                if tot == 0:
                    continue
                sl_full = np.full(tot, -1, dtype=np.int16)
                dl_full = np.full(tot, -1, dtype=np.int16)
                s0 = 0
                for t in b["tiles"]:
                    btq = int(self.B[t, q])
                    if btq == 0:
                        continue
                    n = 0
                    if (t, q) in g:
                        sl, dl = g[(t, q)]
                        n = len(sl)
                        sl_full[s0 : s0 + n] = sl
                        dl_full[s0 : s0 + n] = dl
                    nv = max(n, min(16, btq))
                    sl_full[s0 + n : s0 + nv] = 0
                    cnt[t, q] = nv
                    s0 += btq
                blk = sl_full.reshape(tot // 16, 16).T
                idx16[:, icol : icol + tot // 16] = np.tile(blk, (8, 1))
                icol += tot // 16
                col0 = b["cgo"] + qq["o"]
                nch = qq["nch"]
                dstloc[:, col0 : col0 + nch] = (
                    dl_full.reshape(nch, P).T.astype(np.float16)
                )
        return idx16, dstloc, cnt


class Plan:
    def __init__(self, cfg, src, dst):
        c = cfg
        self.cfg = c
        core = dst // c.SHARD
        e1, e2 = [], []
        for ci in range(c.NCORE):
            m = core == ci
            s = src[m]
            d = dst[m] - ci * c.SHARD
            t = d // P
            dl = d % P
            # layer 1: x16 table grouped by quarters of node id
            e1.append((t, s // c.QS, s % c.QS, dl))
            # layer 2: z_full is [slice][core][row]; slice = shard-row/SROW
            sc = s // c.SHARD
            sr = s % c.SHARD
            sl = sr // c.SROW
            lidx = sc * c.SROW + (sr - sl * c.SROW)
            e2.append((t, sl, lidx, dl))
        self.L1 = LayerPlan(cfg, c.NQ, e1)
        self.L2 = LayerPlan(cfg, c.NS, e2)

    def core_arrays(self, ci):
        i1, d1, c1 = self.L1.core_arrays(ci)
        i2, d2, c2 = self.L2.core_arrays(ci)
        cnts = np.concatenate([c1.reshape(-1), c2.reshape(-1)])
        cnt16 = np.broadcast_to(
            cnts.astype("<i4").view("<i2"), (P, 2 * len(cnts))
        )
        return np.concatenate(
            [i1, d1.view(np.int16), i2, d2.view(np.int16), cnt16], axis=1
        )


def _build(plan):
    import concourse.tile as tile
    from concourse import bacc, mybir

    c = plan.cfg
    f16, f32, i16 = mybir.dt.float16, mybir.dt.float32, mybir.dt.int16
    CH, H, QS, SHARD = c.CH, c.H, c.QS, c.SHARD
    L1, L2 = plan.L1, plan.L2
    CNTW = 2 * (c.NT * (c.NQ + c.NS))
    PKW = L1.IGOT + L1.CGOT + L2.IGOT + L2.CGOT + CNTW
    i32 = mybir.dt.int32

    nc = bacc.Bacc(
        "TRN2", target_bir_lowering=False, debug=False, num_devices=c.NCORE,
        num_swdge_queues=4, dynamic_dma_scratch_size=32768,
    )
    x16 = nc.dram_tensor("x16", [c.N, CH], f16, kind="ExternalInput")
    xT = nc.dram_tensor("xT", [CH, SHARD], f32, kind="ExternalInput")
    pk = nc.dram_tensor("pk", [P, PKW], i16, kind="ExternalInput")
    iota = nc.dram_tensor("iota", [P, P], f16, kind="ExternalInput")
    ident = nc.dram_tensor("ident", [H, H], f16, kind="ExternalInput")
    w1 = nc.dram_tensor("w1", [CH, H], f16, kind="ExternalInput")
    w2 = nc.dram_tensor("w2", [H, H], f16, kind="ExternalInput")
    w3 = nc.dram_tensor("w3", [H, 16], f16, kind="ExternalInput")
    w4 = nc.dram_tensor("w4", [16, 1], f16, kind="ExternalInput")
    b1 = nc.dram_tensor("b1", [H, 1], f32, kind="ExternalInput")
    b2 = nc.dram_tensor("b2", [H, 1], f32, kind="ExternalInput")
    b3 = nc.dram_tensor("b3", [16, 1], f32, kind="ExternalInput")
    b4v = nc.dram_tensor("b4v", [1, 1], f32, kind="ExternalInput")
    out = nc.dram_tensor("out", [1, SHARD], f32, kind="ExternalOutput")

    with tile.TileContext(nc) as tc:
        with (
            tc.tile_pool(name="const", bufs=1) as cp,
            tc.tile_pool(name="persist", bufs=1) as pp,
            tc.tile_pool(name="dram", bufs=1, space="DRAM") as dp,
        ):
            def load_const(name, t, shape, dt):
                sb = cp.tile(shape, dt, name=name + "_sb")
                nc.sync.dma_start(out=sb[:], in_=t[:, :])
                return sb

            iota_sb = load_const("iota", iota, [P, P], f16)
            id_sb = load_const("id", ident, [H, H], f16)
            w1_sb = load_const("w1", w1, [CH, H], f16)
            w2_sb = load_const("w2", w2, [H, H], f16)
            w3_sb = load_const("w3", w3, [H, 16], f16)
            w4_sb = load_const("w4", w4, [16, 1], f16)
            b1_sb = load_const("b1", b1, [H, 1], f32)
            b2_sb = load_const("b2", b2, [H, 1], f32)
            b3_sb = load_const("b3", b3, [16, 1], f32)
            b4_sb = load_const("b4", b4v, [1, 1], f32)

            cnt_sb = pp.tile([P, CNTW], i16)
            nc.sync.dma_start(
                out=cnt_sb[:], in_=pk[:, PKW - CNTW : PKW])
            zT_sh = pp.tile([H, SHARD], f32)
            outT = pp.tile([1, SHARD], f32)
            z_shard = dp.tile([SHARD, P], f16)
            z_full = dp.tile([c.N, P], f16)  # [slice][core][row][ch]

            relu = mybir.ActivationFunctionType.Relu
            qrr = [0]  # round-robin SWDGE queue
            state = {}
            cnt_regs = [nc.gpsimd.alloc_register(f"cntr{i}")
                        for i in range(8)]
            crr = [0]

            def tile_cols(t):
                return min(P, SHARD - t * P)

            def gather_batch(lp, b, pk_base, gbuf, table, tbase_of, elem,
                             cnt_base):
                sm = state["sm"]
                icols = sum(q["icols"] for q in b["quarters"])
                nch = b["nch"]
                pk_sb = sm.tile([P, icols + nch], i16, tag="pk")
                nc.sync.dma_start(
                    out=pk_sb[:, :icols],
                    in_=pk[:, pk_base + b["igo"] : pk_base + b["igo"] + icols],
                )
                nc.sync.dma_start(
                    out=pk_sb[:, icols:],
                    in_=pk[
                        :,
                        pk_base + lp.IGOT + b["cgo"] :
                        pk_base + lp.IGOT + b["cgo"] + nch,
                    ],
                )
                ic = 0
                for q, qq in enumerate(b["quarters"]):
                    if qq["tot"] == 0:
                        continue
                    t0, t1 = tbase_of(q)
                    s0 = 0
                    for t in b["tiles"]:
                        btq = int(lp.B[t, q])
                        if btq == 0:
                            continue
                        co = (cnt_base + t * lp.NGR + q) * 2
                        reg = cnt_regs[crr[0] % 8]
                        crr[0] += 1
                        nc.gpsimd.reg_load(
                            reg, cnt_sb[0:1, co : co + 2].bitcast(i32))
                        c0 = qq["o"] + s0 // P
                        nc.gpsimd.dma_gather(
                            gbuf[:, c0 : c0 + btq // P, :],
                            table[t0:t1, :],
                            pk_sb[:, ic + s0 // 16 : ic + (s0 + btq) // 16],
                            btq, reg, elem,
                            queue_num=qrr[0] % 4,
                        )
                        qrr[0] += 1
                        s0 += btq
                    ic += qq["icols"]
                return pk_sb[:, icols:].bitcast(f16)

            def onehot_and_agg(b, dl_sb, gbuf, ohp, psa, m_dim, tag):
                nch = b["nch"]
                oh = ohp.tile([P, nch, P], f16, tag="oh", name="oh")
                nc.vector.tensor_tensor(
                    out=oh[:],
                    in0=dl_sb.unsqueeze(2).to_broadcast([P, nch, P]),
                    in1=iota_sb[:].unsqueeze(1).to_broadcast([P, nch, P]),
                    op=mybir.AluOpType.is_equal,
                )
                mm = []
                for q, qq in enumerate(b["quarters"]):
                    for (t, ch) in qq["segs"]:
                        mm.append((t, qq["o"] + ch))
                first, last, aggs = {}, {}, {}
                for i, (t, ch) in enumerate(mm):
                    first.setdefault(t, i)
                    last[t] = i
                for t in b["tiles"]:
                    aggs[t] = psa.tile(
                        [m_dim, P], f32, tag=tag, name=f"{tag}_{t}"
                    )
                for i, (t, ch) in enumerate(mm):
                    nc.tensor.matmul(
                        out=aggs[t][:],
                        lhsT=gbuf[:, ch : ch + 1, :],
                        rhs=oh[:, ch : ch + 1, :],
                        start=(i == first[t]),
                        stop=(i == last[t]),
                    )
                return aggs

            # ---------------- phase 1 ----------------
            with (
                tc.tile_pool(name="l1g", bufs=2) as gp,
                tc.tile_pool(name="l1oh", bufs=2) as ohp,
                tc.tile_pool(name="l1sm", bufs=3) as sm,
                tc.tile_pool(name="l1ps", bufs=c.T_B + 1, space="PSUM") as psa,
                tc.tile_pool(name="l1pst", bufs=1, space="PSUM") as pst,
            ):
                state["sm"] = sm
                ag_row = [0]

                def issue_ag(r1):
                    # one collective per completed slice: the AG output
                    # (concat over cores) is contiguous only within a slice
                    r1 = min(r1, SHARD)
                    while ag_row[0] + c.SROW <= r1:
                        r0 = ag_row[0]
                        rs = r0 + c.SROW
                        nc.gpsimd.collective_compute(
                            "AllGather",
                            mybir.AluOpType.bypass,
                            replica_groups=[list(range(c.NCORE))],
                            ins=[z_shard[r0:rs, :].opt()],
                            outs=[
                                z_full[r0 * c.NCORE : rs * c.NCORE, :].opt()
                            ],
                        )
                        ag_row[0] = rs

                for b in L1.batches:
                    if b["nch"] == 0:
                        continue
                    g1 = gp.tile([P, b["nch"], CH], f16, tag="g1", name="g1")
                    dl_sb = gather_batch(
                        L1, b, 0, g1, x16,
                        lambda q: (q * QS, (q + 1) * QS), CH, 0,
                    )
                    aggs = onehot_and_agg(b, dl_sb, g1, ohp, psa, CH, "agg1")
                    for t in b["tiles"]:
                        tw = tile_cols(t)
                        xT_sb = sm.tile([CH, P], f32, tag="xt", name="xt")
                        nc.scalar.dma_start(
                            out=xT_sb[:, :tw], in_=xT[:, t * P : t * P + tw]
                        )
                        sT = sm.tile([CH, P], f16, tag="st", name="st")
                        nc.vector.tensor_add(
                            out=sT[:, :tw], in0=aggs[t][:, :tw],
                            in1=xT_sb[:, :tw],
                        )
                        h1p = pst.tile([H, P], f32, tag="h1", name="h1p")
                        nc.tensor.matmul(
                            out=h1p[:, :tw], lhsT=w1_sb[:], rhs=sT[:, :tw],
                            start=True, stop=True,
                        )
                        h1f = sm.tile([H, P], f16, tag="h1f", name="h1f")
                        nc.scalar.activation(
                            out=h1f[:, :tw], in_=h1p[:, :tw], func=relu,
                            bias=b1_sb[:],
                        )
                        zp = pst.tile([H, P], f32, tag="zp", name="zp")
                        nc.tensor.matmul(
                            out=zp[:, :tw], lhsT=w2_sb[:], rhs=h1f[:, :tw],
                            start=True, stop=True,
                        )
                        nc.vector.tensor_copy(
                            out=zT_sh[:, t * P : t * P + tw], in_=zp[:, :tw]
                        )
                        zf = sm.tile([H, P], f16, tag="zf", name="zf")
                        nc.vector.tensor_copy(out=zf[:, :tw], in_=zp[:, :tw])
                        ztr = pst.tile([P, H], f16, tag="ztr", name="ztr")
                        nc.tensor.transpose(
                            out=ztr[:tw, :], in_=zf[:, :tw], identity=id_sb[:]
                        )
                        zr = sm.tile([P, P], f16, tag="zr", name="zr")
                        nc.vector.memset(zr[:, H:], 0)
                        nc.vector.tensor_copy(out=zr[:tw, :H], in_=ztr[:tw, :])
                        nc.scalar.dma_start(
                            out=z_shard[t * P : t * P + tw, :], in_=zr[:tw, :]
                        )
                    done_rows = (b["tiles"][-1] + 1) * P
                    issue_ag((done_rows // c.SROW) * c.SROW)
                issue_ag(SHARD)

            # ---------------- phase 2 ----------------
            pk2 = L1.IGOT + L1.CGOT
            with (
                tc.tile_pool(name="l2g", bufs=2) as gp,
                tc.tile_pool(name="l2oh", bufs=2) as ohp,
                tc.tile_pool(name="l2sm", bufs=3) as sm,
                tc.tile_pool(name="l2ps", bufs=c.T_B + 1, space="PSUM") as psa,
                tc.tile_pool(name="l2pst", bufs=1, space="PSUM") as pst,
            ):
                state["sm"] = sm
                SR8 = c.SROW * c.NCORE
                for b in L2.batches:
                    if b["nch"] == 0:
                        continue
                    g2 = gp.tile([P, b["nch"], P], f16, tag="g2", name="g2")
                    dl_sb = gather_batch(
                        L2, b, pk2, g2, z_full,
                        lambda s: (s * SR8, (s + 1) * SR8), P,
                        c.NT * c.NQ,
                    )
                    aggs = onehot_and_agg(b, dl_sb, g2, ohp, psa, P, "agg2")
                    for t in b["tiles"]:
                        tw = tile_cols(t)
                        t2 = sm.tile([H, P], f32, tag="t2", name="t2")
                        nc.vector.tensor_add(
                            out=t2[:, :tw], in0=aggs[t][:H, :tw],
                            in1=zT_sh[:, t * P : t * P + tw],
                        )
                        h2f = sm.tile([H, P], f16, tag="h2f", name="h2f")
                        nc.scalar.activation(
                            out=h2f[:, :tw], in_=t2[:, :tw], func=relu,
                            bias=b2_sb[:],
                        )
                        h3p = pst.tile([16, P], f32, tag="h3", name="h3p")
                        nc.tensor.matmul(
                            out=h3p[:, :tw], lhsT=w3_sb[:], rhs=h2f[:, :tw],
                            start=True, stop=True,
                        )
                        h3f = sm.tile([16, P], f16, tag="h3f", name="h3f")
                        nc.scalar.activation(
                            out=h3f[:, :tw], in_=h3p[:, :tw], func=relu,
                            bias=b3_sb[:],
                        )
                        op_ = pst.tile([1, P], f32, tag="op", name="op_")
                        nc.tensor.matmul(
                            out=op_[:, :tw], lhsT=w4_sb[:], rhs=h3f[:, :tw],
                            start=True, stop=True,
                        )
                        nc.vector.scalar_tensor_tensor(
                            out=outT[:, t * P : t * P + tw],
                            in0=op_[:, :tw],
                            scalar=1.0,
                            in1=b4_sb[:].to_broadcast([1, tw]),
                            op0=mybir.AluOpType.mult,
                            op1=mybir.AluOpType.add,
                        )
            nc.sync.dma_start(out=out[:, :], in_=outT[:])
    nc.compile()
    return nc


def _in_maps(cfg, plan, x, W1, b1, W2, b2, W3, b3, W4, b4):
    c = cfg
    common = dict(
        x16=x.astype(np.float16),
        iota=np.broadcast_to(np.arange(P, dtype=np.float16), (P, P)).copy(),
        ident=np.eye(c.H, dtype=np.float16),
        w1=W1.astype(np.float16),
        w2=W2.astype(np.float16),
        w3=W3.astype(np.float16),
        w4=W4.astype(np.float16),
        b1=b1.reshape(-1, 1).astype(np.float32),
        b2=b2.reshape(-1, 1).astype(np.float32),
        b3=b3.reshape(-1, 1).astype(np.float32),
        b4v=b4.reshape(1, 1).astype(np.float32),
    )
    in_maps = []
    for ci in range(c.NCORE):
        pk_a = plan.core_arrays(ci)
        xT_a = np.ascontiguousarray(
            x[ci * c.SHARD : (ci + 1) * c.SHARD].T.astype(np.float32)
        )
        in_maps.append(dict(common, pk=pk_a, xT=xT_a))
    return in_maps


def _run(cfg, plan, nc, x, W1, b1, W2, b2, W3, b3, W4, b4, **kw):
    from concourse.bass_utils import run_bass_kernel_spmd

    c = cfg
    in_maps = _in_maps(cfg, plan, x, W1, b1, W2, b2, W3, b3, W4, b4)
    res = run_bass_kernel_spmd(nc, in_maps, core_ids=list(range(c.NCORE)), **kw)
    outs = [res.results[ci]["out"].reshape(-1) for ci in range(c.NCORE)]
    return np.concatenate(outs).reshape(-1, 1).astype(np.float32), res


def kernel(x, edge_index, W1, b1, W2, b2, W3, b3, W4, b4):
    cfg = FULL
    x = np.asarray(x, dtype=np.float32)
    src = np.asarray(edge_index[0], dtype=np.int64)
    dst = np.asarray(edge_index[1], dtype=np.int64)
    plan = Plan(cfg, src, dst)
    nc = _build(plan)
    out, _ = _run(
        cfg, plan, nc, x,
        np.asarray(W1), np.asarray(b1), np.asarray(W2), np.asarray(b2),
        np.asarray(W3), np.asarray(b3), np.asarray(W4), np.asarray(b4),
    )
    return out

